# revision 19
# baseline (speedup 1.0000x reference)
"""Trainium2 Bass kernel for nn_MultiHeadAttention_30167850287106.

Reference computation (B=4, S=1024, D=1024, H=16, HD=64):
    Q = (x @ Wq)  K = (x @ Wk)  V = (x @ Wv)     per-head [b,h,s,hd]
    scores = (Q * mixing) K^T / sqrt(hd) + (x @ Wb)^T + mask
    P = softmax(scores);  out = (P V) @ Wd + bd;  returns (out, P)

Sharding: 8 cores = (batch b, query-half qh).  Each core owns queries
[qh*512, qh*512+512) of batch b and produces fully disjoint output slices
(no collectives).  K/V projections are duplicated across the pair.

Per-core data flow (scores matmuls in float32r = full PE rate, ~1e-4 rel):
  Host stages xT = x[b]^T and folds mixing/sqrt(hd) into Wq.
  Augmented per-head tiles:
     QT_h [66, 512]:  rows 0:64 = (mix*Q)^T, row 64 = ones, row 65 = ln(1/denom)
     KT_h [66, 1024]: rows 0:64 = K^T,      row 64 = cb+mask, row 65 = ones
  pass A (rows 0:65): S[q,k] -> exp (ACT accum row sums) -> P/denom -> HBM
  pass B (rows 0:66): S^T[k,q] + ln(1/denom) -> exp = normalized P^T (bf16)
  O^T = V^T P^T accumulated in PSUM (col-tiled per head pair) -> OT (bf16)
  dense: out = OT^T Wd (bf16) -> HBM.
  The K projection is interleaved with the per-head attention pipeline so the
  ACT-bound attention work starts as soon as the first K m-tile lands.
"""

import numpy as np
import ml_dtypes

from concourse import bacc, tile, mybir
from concourse.bass_utils import run_bass_kernel_spmd

# Both Exp and Ln live in the "natural_log_exp_and_others" ACT table set, but
# the default per-function set choice alternates exp_and_others <-> natural_log
# per head (33 table loads, ~2.7us each).  Empty every other set (keeping list
# positions, which are the act_func_set_ids) so one resident set serves both.
_PREFERRED_ACT_SET = "natural_log_exp_and_others"
_orig_get_act_tables = bacc.get_activation_tables


def _patched_get_act_tables(arch):
    tables = dict(_orig_get_act_tables(arch))
    return {k: (v if k == _PREFERRED_ACT_SET else set()) for k, v in tables.items()}


bacc.get_activation_tables = _patched_get_act_tables

AF = mybir.ActivationFunctionType
F32 = mybir.dt.float32
F32R = mybir.dt.float32r
BF16 = mybir.dt.bfloat16

B, S, D, H, HD = 4, 1024, 1024, 16, 64
SQ = 512  # queries per core

_cache = {}
LAST_RESULTS = None


def _head_A(nc, tc, h, KT, QT, ppool, psA, smp, attw):
    """Pass A for head h: S[q,k] -> exp/sums -> normalized P -> HBM,
    and ln(1/denom) into QT row 65 for pass B."""
    sums = smp.tile([128, 4], F32, tag="sums", name=f"sums{h}")
    recip = smp.tile([128, 4], F32, tag="recip", name=f"recip{h}")
    lnr = smp.tile([128, 4], F32R, tag="lnr", name=f"lnr{h}")
    for qt in range(4):
        ps = psA.tile([128, 1024], F32, tag="psA", name=f"psa{h}_{qt}")
        for n in range(2):
            sl = slice(n * 512, (n + 1) * 512)
            nc.tensor.matmul(
                ps[:, sl],
                QT[h][0:65, qt * 128:(qt + 1) * 128],
                KT[h][0:65, sl],
                start=True, stop=True)
        p_ = ppool.tile([128, 1024], F32, tag="p", name=f"p{h}_{qt}")
        nc.scalar.activation(
            out=p_[:], in_=ps[:], func=AF.Exp, accum_out=sums[:, qt:qt + 1])
        nc.vector.reciprocal(recip[:, qt:qt + 1], sums[:, qt:qt + 1])
        nc.vector.tensor_scalar_mul(
            out=p_[:], in0=p_[:], scalar1=recip[:, qt:qt + 1])
        nc.sync.dma_start(
            out=attw[h, qt * 128:(qt + 1) * 128, :], in_=p_[:])
    # +ln(denom) paired with the -1 row of KT gives -ln(denom) in pass B
    nc.scalar.activation(out=lnr[:], in_=sums[:], func=AF.Ln)
    for qt in range(4):
        nc.sync.dma_start(
            out=QT[h][65:66, qt * 128:(qt + 1) * 128], in_=lnr[:, qt:qt + 1])


def _pair_BO(nc, tc, j, KT, QT, V, OT, ptp, psB, psO):
    """Pass B + O^T for head pair j (heads 2j, 2j+1)."""
    pso = psO.tile([128, 512], F32, tag="psO", name=f"pso{j}")
    for hh in range(2):
        h = 2 * j + hh
        pt_list = []
        for kt in range(8):
            psb = psB.tile([128, 512], F32, tag="psB", name=f"psb{h}_{kt}")
            nc.tensor.matmul(
                psb[:],
                KT[h][0:66, kt * 128:(kt + 1) * 128],
                QT[h][0:66, :],
                start=True, stop=True)
            ptt = ptp.tile([128, 512], BF16, tag="pt", name=f"pt{h}_{kt}")
            nc.scalar.activation(out=ptt[:], in_=psb[:], func=AF.Exp)
            pt_list.append(ptt)
        for kt in range(8):
            nc.tensor.matmul(
                pso[hh * 64:(hh + 1) * 64, :],
                V[kt][:, h * 64:(h + 1) * 64],
                pt_list[kt][:],
                start=(kt == 0), stop=(kt == 7),
                tile_position=(0, 64 * hh))
    nc.vector.tensor_copy(OT[j][:], pso[:])


def build(nreps=1):
    if ("nc", nreps) in _cache:
        return _cache[("nc", nreps)]
    nc = bacc.Bacc("TRN2", target_bir_lowering=False, debug=False, num_devices=8)

    xT = nc.dram_tensor("xT", [D, S], F32R, kind="ExternalInput")
    xq = nc.dram_tensor("xq", [D, SQ], F32R, kind="ExternalInput")
    wq = nc.dram_tensor("wq", [D, D], F32R, kind="ExternalInput")  # pre-scaled
    wk = nc.dram_tensor("wk", [D, D], F32R, kind="ExternalInput")
    wv = nc.dram_tensor("wv", [D, D], F32R, kind="ExternalInput")
    wb = nc.dram_tensor("wb", [D, H], F32R, kind="ExternalInput")
    wd = nc.dram_tensor("wd", [D, D], BF16, kind="ExternalInput")
    mask = nc.dram_tensor("mask", [1, S], F32R, kind="ExternalInput")
    attw = nc.dram_tensor("attw", [H, SQ, S], F32, kind="ExternalOutput")
    atto = nc.dram_tensor("atto", [SQ, D], F32, kind="ExternalOutput")

    ones_dram = nc.inline_tensor(np.ones((1, SQ), np.float32), name="ones_c")
    mones_dram = nc.inline_tensor(np.full((1, SQ), -1.0, np.float32), name="mones_c")

    with tile.TileContext(nc) as tc:
      for _rep in range(nreps):
        with (
            tc.tile_pool(name="aug", bufs=1) as augp,
            tc.tile_pool(name="vp", bufs=1) as vp,
            tc.tile_pool(name="otp", bufs=1) as otp,
        ):
            KT = [augp.tile([66, S], F32R, name=f"kt{h}") for h in range(H)]
            QT = [augp.tile([66, SQ], F32R, name=f"qt{h}") for h in range(H)]
            V = [vp.tile([128, D], BF16, name=f"v{k}") for k in range(8)]
            OT = [otp.tile([128, SQ], BF16, name=f"ot{j}") for j in range(8)]
            for h in range(H):
                nc.sync.dma_start(
                    out=QT[h][64:65, :], in_=ones_dram[:].bitcast(F32R))
                nc.sync.dma_start(
                    out=KT[h][65:66, 0:SQ], in_=mones_dram[:].bitcast(F32R))
                nc.sync.dma_start(
                    out=KT[h][65:66, SQ:S], in_=mones_dram[:].bitcast(F32R))

            with (
                tc.tile_pool(name="xt", bufs=1) as xtp,
            ):
                xt = [xtp.tile([128, S], F32R, name=f"xt{k}") for k in range(8)]
                for k in range(8):
                    nc.sync.dma_start(out=xt[k][:], in_=xT[k * 128:(k + 1) * 128, :])

                # ---- content bias ----
                with (
                    tc.tile_pool(name="cst", bufs=1) as cst,
                    tc.tile_pool(name="ppcb", bufs=1, space="PSUM") as ppcb,
                ):
                    # cball row 32 = mask (base-32 so it can be a matmul operand)
                    cball = cst.tile([33, S], F32R, name="cball")
                    nc.sync.dma_start(out=cball[32:33, :], in_=mask[:])
                    ones16 = cst.tile([33, H], F32R, name="ones16")
                    nc.sync.dma_start(
                        out=ones16[32:33, :], in_=ones_dram[:, 0:H].bitcast(F32R))
                    wbt = cst.tile([128, 8, H], F32R, name="wbt")
                    for k in range(8):
                        nc.sync.dma_start(
                            out=wbt[:, k, :], in_=wb[k * 128:(k + 1) * 128, :])
                    cbps = ppcb.tile([16, 1024], F32, name="cbps")
                    for n in range(2):
                        sl = slice(n * 512, (n + 1) * 512)
                        for k in range(8):
                            nc.tensor.matmul(
                                cbps[:, sl], wbt[:, k, :], xt[k][:, sl],
                                start=(k == 0), stop=False)
                        nc.tensor.matmul(
                            cbps[:, sl], ones16[32:33, :], cball[32:33, sl],
                            start=False, stop=True)
                    nc.vector.tensor_copy(cball[0:16, :], cbps[:])
                    for h in range(H):
                        nc.sync.dma_start(
                            out=KT[h][64:65, :], in_=cball[h:h + 1, :])

                # ---- Q^T (wq pre-scaled by mixing/sqrt(hd)) ----
                with (
                    tc.tile_pool(name="wqp", bufs=8) as wqp,
                    tc.tile_pool(name="xqp", bufs=1) as xqp,
                    tc.tile_pool(name="scrq", bufs=2) as scrq,
                    tc.tile_pool(name="ppsQ", bufs=4, space="PSUM") as ppsQ,
                ):
                    xqt = [xqp.tile([128, SQ], F32R, name=f"xq{k}")
                           for k in range(8)]
                    for k in range(8):
                        nc.sync.dma_start(
                            out=xqt[k][:], in_=xq[k * 128:(k + 1) * 128, :])
                    wq_t = [wqp.tile([128, D], F32R, tag="wq", name=f"wq{k}")
                            for k in range(8)]
                    for k in range(8):
                        nc.sync.dma_start(
                            out=wq_t[k][:], in_=wq[k * 128:(k + 1) * 128, :])
                    for m in range(8):
                        ps = ppsQ.tile([128, 512], F32, tag="pps", name=f"psq{m}")
                        for k in range(8):
                            nc.tensor.matmul(
                                ps[:], wq_t[k][:, m * 128:(m + 1) * 128],
                                xqt[k][:], start=(k == 0), stop=(k == 7))
                        sc = scrq.tile([128, 512], F32R, tag="scr", name=f"scq{m}")
                        nc.vector.tensor_copy(sc[:], ps[:])
                        nc.sync.dma_start(out=QT[2 * m][0:64, :], in_=sc[0:64, :])
                        nc.sync.dma_start(
                            out=QT[2 * m + 1][0:64, :], in_=sc[64:128, :])

                # wk prefetch pool opens as soon as Q's pools free
                with tc.tile_pool(name="wkp", bufs=5) as wkp:
                    wk_t = [wkp.tile([128, D], F32R, tag="wk", name=f"wk{k}")
                            for k in range(5)]
                    for k in range(5):
                        nc.sync.dma_start(
                            out=wk_t[k][:], in_=wk[k * 128:(k + 1) * 128, :])

                    # ---- V (natural layout [s, d']) ----
                    with (
                        tc.tile_pool(name="wvp", bufs=8) as wvp,
                        tc.tile_pool(name="ppsV", bufs=4, space="PSUM") as ppsV,
                    ):
                        wv_t = [wvp.tile([128, D], F32R, tag="wv", name=f"wv{k}")
                                for k in range(8)]
                        for k in range(8):
                            nc.sync.dma_start(
                                out=wv_t[k][:], in_=wv[k * 128:(k + 1) * 128, :])
                        for m in range(8):
                            for n in range(2):
                                sl = slice(n * 512, (n + 1) * 512)
                                ps = ppsV.tile([128, 512], F32, tag="pps",
                                               name=f"psv{m}{n}")
                                for k in range(8):
                                    nc.tensor.matmul(
                                        ps[:], xt[k][:, m * 128:(m + 1) * 128],
                                        wv_t[k][:, sl], start=(k == 0), stop=(k == 7))
                                nc.vector.tensor_copy(V[m][:, sl], ps[:])

                    # ---- interleaved K projection + attention pipeline ----
                    with (
                        tc.tile_pool(name="wk7p", bufs=1) as wk7p,
                        tc.tile_pool(name="scrk", bufs=2) as scrk,
                        tc.tile_pool(name="pp", bufs=3) as ppool,
                        tc.tile_pool(name="ptp", bufs=6) as ptp,
                        tc.tile_pool(name="smp", bufs=2) as smp,
                        tc.tile_pool(name="ppsK", bufs=1, space="PSUM") as ppsK,
                        tc.tile_pool(name="psA", bufs=2, space="PSUM") as psA,
                        tc.tile_pool(name="psB", bufs=2, space="PSUM") as psB,
                        tc.tile_pool(name="psO", bufs=1, space="PSUM") as psO,
                    ):
                        wk_late = [wk7p.tile([128, D], F32R, name=f"wk{k}l")
                                   for k in (5, 6, 7)]
                        for i, k in enumerate((5, 6, 7)):
                            nc.sync.dma_start(
                                out=wk_late[i][:], in_=wk[k * 128:(k + 1) * 128, :])
                        wk_all = wk_t + wk_late
                        prev = None
                        for m in range(8):
                            if prev is not None:
                                _pair_BO(nc, tc, prev, KT, QT, V, OT, ptp, psB, psO)
                            for n in range(2):
                                sl = slice(n * 512, (n + 1) * 512)
                                ps = ppsK.tile([128, 512], F32, tag="ppsK",
                                               name=f"psk{m}{n}")
                                for k in range(8):
                                    nc.tensor.matmul(
                                        ps[:], wk_all[k][:, m * 128:(m + 1) * 128],
                                        xt[k][:, sl], start=(k == 0), stop=(k == 7))
                                sc = scrk.tile([128, 512], F32R, tag="scr",
                                               name=f"sck{m}{n}")
                                nc.vector.tensor_copy(sc[:], ps[:])
                                nc.sync.dma_start(
                                    out=KT[2 * m][0:64, sl], in_=sc[0:64, :])
                                nc.sync.dma_start(
                                    out=KT[2 * m + 1][0:64, sl], in_=sc[64:128, :])
                            _head_A(nc, tc, 2 * m, KT, QT, ppool, psA, smp, attw)
                            _head_A(nc, tc, 2 * m + 1, KT, QT, ppool, psA, smp, attw)
                            prev = m
                        _pair_BO(nc, tc, prev, KT, QT, V, OT, ptp, psB, psO)

            # ---- dense output projection (bf16) ----
            with (
                tc.tile_pool(name="wdp", bufs=8) as wdp,
                tc.tile_pool(name="dop", bufs=3) as dop,
                tc.tile_pool(name="psD", bufs=4, space="PSUM") as psD,
            ):
                wd_t = [wdp.tile([128, D], BF16, tag="wd", name=f"wd{k}")
                        for k in range(8)]
                for k in range(8):
                    nc.sync.dma_start(out=wd_t[k][:], in_=wd[k * 128:(k + 1) * 128, :])
                for m in range(4):
                    for n in range(2):
                        sl = slice(n * 512, (n + 1) * 512)
                        ps = psD.tile([128, 512], F32, tag="psD", name=f"psd{m}{n}")
                        for k in range(8):
                            nc.tensor.matmul(
                                ps[:],
                                OT[k][:, m * 128:(m + 1) * 128],
                                wd_t[k][:, sl],
                                start=(k == 0), stop=(k == 7))
                        ot_ = dop.tile([128, 512], F32, tag="do", name=f"do{m}{n}")
                        nc.vector.tensor_copy(ot_[:], ps[:])
                        nc.sync.dma_start(
                            out=atto[m * 128:(m + 1) * 128, sl], in_=ot_[:])

    nc.compile()
    _cache[("nc", nreps)] = nc
    return nc


def make_in_maps(x, attention_mask, Wq, Wk, Wv, Wb, mixing, Wd):
    x = np.asarray(x, dtype=np.float32)
    mixflat = np.asarray(mixing, dtype=np.float32).reshape(-1) / np.float32(np.sqrt(HD))
    wq_eff = np.ascontiguousarray(np.asarray(Wq, np.float32) * mixflat[None, :])
    wk_ = np.ascontiguousarray(np.asarray(Wk, np.float32))
    wv_ = np.ascontiguousarray(np.asarray(Wv, np.float32))
    wb_ = np.ascontiguousarray(np.asarray(Wb, np.float32))
    wd_ = np.ascontiguousarray(np.asarray(Wd, np.float32).astype(ml_dtypes.bfloat16))
    am = np.asarray(attention_mask, np.float32)
    in_maps = []
    for c in range(8):
        b, qh = c // 2, c % 2
        xTb = np.ascontiguousarray(x[b].T)
        in_maps.append({
            "xT": xTb,
            "xq": np.ascontiguousarray(xTb[:, qh * SQ:(qh + 1) * SQ]),
            "wq": wq_eff, "wk": wk_, "wv": wv_, "wb": wb_, "wd": wd_,
            "mask": np.ascontiguousarray(am[b, 0, 0, :].reshape(1, S)),
        })
    return in_maps


def kernel(x, attention_mask, Wq, Wk, Wv, Wb, mixing, Wd, bd, _trace=False):
    global LAST_RESULTS
    nc = build()
    in_maps = make_in_maps(x, attention_mask, Wq, Wk, Wv, Wb, mixing, Wd)
    res = run_bass_kernel_spmd(nc, in_maps, list(range(8)), trace=_trace)
    LAST_RESULTS = res
    attn_output = np.empty((B, S, D), np.float32)
    attn_weights = np.empty((B, H, S, S), np.float32)
    for c in range(8):
        b, qh = c // 2, c % 2
        r = res.results[c]
        attn_output[b, qh * SQ:(qh + 1) * SQ, :] = r["atto"]
        attn_weights[b, :, qh * SQ:(qh + 1) * SQ, :] = r["attw"]
    attn_output += np.asarray(bd, np.float32)[None, None, :]
    return attn_output, attn_weights


# revision 23
# speedup vs baseline: 4.3421x; 4.3421x over previous
"""Trainium2 Bass kernel for nn_MultiHeadAttention_30167850287106.

Reference computation (B=4, S=1024, D=1024, H=16, HD=64):
    Q = (x @ Wq)  K = (x @ Wk)  V = (x @ Wv)     per-head [b,h,s,hd]
    scores = (Q * mixing) K^T / sqrt(hd) + (x @ Wb)^T + mask
    P = softmax(scores);  out = (P V) @ Wd + bd;  returns (out, P)

Sharding: 8 cores = (batch b, query-half qh).  Each core owns queries
[qh*512, qh*512+512) of batch b and produces fully disjoint output slices
(no collectives).  K/V projections are duplicated across the pair.

Per-core data flow (scores matmuls in float32r = full PE rate, ~1e-4 rel):
  Host stages xT = x[b]^T and folds mixing/sqrt(hd) into Wq.
  Augmented per-head tiles:
     QT_h [66, 512]:  rows 0:64 = (mix*Q)^T, row 64 = ones, row 65 = ln(1/denom)
     KT_h [66, 1024]: rows 0:64 = K^T,      row 64 = cb+mask, row 65 = ones
  pass A (rows 0:65): S[q,k] -> exp (ACT accum row sums) -> P/denom -> HBM
  pass B (rows 0:66): S^T[k,q] + ln(1/denom) -> exp = normalized P^T (bf16)
  O^T = V^T P^T accumulated in PSUM (col-tiled per head pair) -> OT (bf16)
  dense: out = OT^T Wd (bf16) -> HBM.
  The K projection is interleaved with the per-head attention pipeline so the
  ACT-bound attention work starts as soon as the first K m-tile lands.
"""

import numpy as np
import ml_dtypes

from concourse import bacc, tile, mybir
from concourse.bass_utils import run_bass_kernel_spmd

# Both Exp and Ln live in the "natural_log_exp_and_others" ACT table set, but
# the default per-function set choice alternates exp_and_others <-> natural_log
# per head (33 table loads, ~2.7us each).  Empty every other set (keeping list
# positions, which are the act_func_set_ids) so one resident set serves both.
_PREFERRED_ACT_SET = "natural_log_exp_and_others"
_orig_get_act_tables = bacc.get_activation_tables


def _patched_get_act_tables(arch):
    tables = dict(_orig_get_act_tables(arch))
    return {k: (v if k == _PREFERRED_ACT_SET else set()) for k, v in tables.items()}


bacc.get_activation_tables = _patched_get_act_tables

AF = mybir.ActivationFunctionType
F32 = mybir.dt.float32
F32R = mybir.dt.float32r
BF16 = mybir.dt.bfloat16

B, S, D, H, HD = 4, 1024, 1024, 16, 64
SQ = 512  # queries per core

_cache = {}
LAST_RESULTS = None


def _head_A(nc, tc, h, KT, QT, ppool, psA, smp, attw):
    """Pass A for head h: S[q,k] -> exp/sums -> normalized P -> HBM,
    and ln(1/denom) into QT row 65 for pass B."""
    sums = smp.tile([128, 4], F32, tag="sums", name=f"sums{h}")
    recip = smp.tile([128, 4], F32, tag="recip", name=f"recip{h}")
    lnr = smp.tile([128, 4], F32R, tag="lnr", name=f"lnr{h}")
    for qt in range(4):
        ps = psA.tile([128, 1024], F32, tag="psA", name=f"psa{h}_{qt}")
        for n in range(2):
            sl = slice(n * 512, (n + 1) * 512)
            nc.tensor.matmul(
                ps[:, sl],
                QT[h][0:65, qt * 128:(qt + 1) * 128],
                KT[h][0:65, sl],
                start=True, stop=True)
        p_ = ppool.tile([128, 1024], F32, tag="p", name=f"p{h}_{qt}")
        nc.scalar.activation(
            out=p_[:], in_=ps[:], func=AF.Exp, accum_out=sums[:, qt:qt + 1])
        nc.vector.reciprocal(recip[:, qt:qt + 1], sums[:, qt:qt + 1])
        nc.vector.tensor_scalar_mul(
            out=p_[:], in0=p_[:], scalar1=recip[:, qt:qt + 1])
        nc.sync.dma_start(
            out=attw[h, qt * 128:(qt + 1) * 128, :], in_=p_[:])
    # +ln(denom) paired with the -1 row of KT gives -ln(denom) in pass B
    nc.scalar.activation(out=lnr[:], in_=sums[:], func=AF.Ln)
    for qt in range(4):
        nc.sync.dma_start(
            out=QT[h][65:66, qt * 128:(qt + 1) * 128], in_=lnr[:, qt:qt + 1])


def _pair_BO(nc, tc, j, KT, QT, V, OT, ptp, psB, psO):
    """Pass B + O^T for head pair j (heads 2j, 2j+1)."""
    pso = psO.tile([128, 512], F32, tag="psO", name=f"pso{j}")
    for hh in range(2):
        h = 2 * j + hh
        pt_list = []
        for kt in range(8):
            psb = psB.tile([128, 512], F32, tag="psB", name=f"psb{h}_{kt}")
            nc.tensor.matmul(
                psb[:],
                KT[h][0:66, kt * 128:(kt + 1) * 128],
                QT[h][0:66, :],
                start=True, stop=True)
            ptt = ptp.tile([128, 512], BF16, tag="pt", name=f"pt{h}_{kt}")
            nc.scalar.activation(out=ptt[:], in_=psb[:], func=AF.Exp)
            pt_list.append(ptt)
        for kt in range(8):
            nc.tensor.matmul(
                pso[hh * 64:(hh + 1) * 64, :],
                V[kt][:, h * 64:(h + 1) * 64],
                pt_list[kt][:],
                start=(kt == 0), stop=(kt == 7),
                tile_position=(0, 64 * hh))
    nc.vector.tensor_copy(OT[j][:], pso[:])


def build(nreps=1):
    if ("nc", nreps) in _cache:
        return _cache[("nc", nreps)]
    nc = bacc.Bacc("TRN2", target_bir_lowering=False, debug=False, num_devices=8)

    xT = nc.dram_tensor("xT", [D, S], F32R, kind="ExternalInput")
    xq = nc.dram_tensor("xq", [D, SQ], F32R, kind="ExternalInput")
    wq = nc.dram_tensor("wq", [D, D], F32R, kind="ExternalInput")  # pre-scaled
    wk = nc.dram_tensor("wk", [D, D], F32R, kind="ExternalInput")
    wv = nc.dram_tensor("wv", [D, D], F32R, kind="ExternalInput")
    wb = nc.dram_tensor("wb", [D, H], F32R, kind="ExternalInput")
    wd = nc.dram_tensor("wd", [D, D], BF16, kind="ExternalInput")
    mask = nc.dram_tensor("mask", [1, S], F32R, kind="ExternalInput")
    attw = nc.dram_tensor("attw", [H, SQ, S], F32, kind="ExternalOutput")
    atto = nc.dram_tensor("atto", [SQ, D], F32, kind="ExternalOutput")

    ones_dram = nc.inline_tensor(np.ones((1, SQ), np.float32), name="ones_c")
    mones_dram = nc.inline_tensor(np.full((1, SQ), -1.0, np.float32), name="mones_c")

    with tile.TileContext(nc) as tc:
      for _rep in range(nreps):
        with (
            tc.tile_pool(name="aug", bufs=1) as augp,
            tc.tile_pool(name="vp", bufs=1) as vp,
            tc.tile_pool(name="otp", bufs=1) as otp,
        ):
            KT = [augp.tile([66, S], F32R, name=f"kt{h}") for h in range(H)]
            QT = [augp.tile([66, SQ], F32R, name=f"qt{h}") for h in range(H)]
            V = [vp.tile([128, D], BF16, name=f"v{k}") for k in range(8)]
            OT = [otp.tile([128, SQ], BF16, name=f"ot{j}") for j in range(8)]
            for h in range(H):
                nc.sync.dma_start(
                    out=QT[h][64:65, :], in_=ones_dram[:].bitcast(F32R))
                nc.sync.dma_start(
                    out=KT[h][65:66, 0:SQ], in_=mones_dram[:].bitcast(F32R))
                nc.sync.dma_start(
                    out=KT[h][65:66, SQ:S], in_=mones_dram[:].bitcast(F32R))

            with (
                tc.tile_pool(name="xt", bufs=1) as xtp,
            ):
                xt = [xtp.tile([128, S], F32R, name=f"xt{k}") for k in range(8)]
                for k in range(8):
                    nc.sync.dma_start(out=xt[k][:], in_=xT[k * 128:(k + 1) * 128, :])

                # ---- content bias ----
                with (
                    tc.tile_pool(name="cst", bufs=1) as cst,
                    tc.tile_pool(name="ppcb", bufs=1, space="PSUM") as ppcb,
                ):
                    # cball row 32 = mask (base-32 so it can be a matmul operand)
                    cball = cst.tile([33, S], F32R, name="cball")
                    nc.sync.dma_start(out=cball[32:33, :], in_=mask[:])
                    ones16 = cst.tile([33, H], F32R, name="ones16")
                    nc.sync.dma_start(
                        out=ones16[32:33, :], in_=ones_dram[:, 0:H].bitcast(F32R))
                    wbt = cst.tile([128, 8, H], F32R, name="wbt")
                    for k in range(8):
                        nc.sync.dma_start(
                            out=wbt[:, k, :], in_=wb[k * 128:(k + 1) * 128, :])
                    cbps = ppcb.tile([16, 1024], F32, name="cbps")
                    for n in range(2):
                        sl = slice(n * 512, (n + 1) * 512)
                        for k in range(8):
                            nc.tensor.matmul(
                                cbps[:, sl], wbt[:, k, :], xt[k][:, sl],
                                start=(k == 0), stop=False)
                        nc.tensor.matmul(
                            cbps[:, sl], ones16[32:33, :], cball[32:33, sl],
                            start=False, stop=True)
                    nc.vector.tensor_copy(cball[0:16, :], cbps[:])
                    for h in range(H):
                        nc.sync.dma_start(
                            out=KT[h][64:65, :], in_=cball[h:h + 1, :])

                # ---- Q^T (wq pre-scaled by mixing/sqrt(hd)) ----
                with (
                    tc.tile_pool(name="wqp", bufs=8) as wqp,
                    tc.tile_pool(name="xqp", bufs=1) as xqp,
                    tc.tile_pool(name="scrq", bufs=2) as scrq,
                    tc.tile_pool(name="ppsQ", bufs=4, space="PSUM") as ppsQ,
                ):
                    xqt = [xqp.tile([128, SQ], F32R, name=f"xq{k}")
                           for k in range(8)]
                    for k in range(8):
                        nc.sync.dma_start(
                            out=xqt[k][:], in_=xq[k * 128:(k + 1) * 128, :])
                    wq_t = [wqp.tile([128, D], F32R, tag="wq", name=f"wq{k}")
                            for k in range(8)]
                    for k in range(8):
                        nc.sync.dma_start(
                            out=wq_t[k][:], in_=wq[k * 128:(k + 1) * 128, :])
                    for m in range(8):
                        ps = ppsQ.tile([128, 512], F32, tag="pps", name=f"psq{m}")
                        for k in range(8):
                            nc.tensor.matmul(
                                ps[:], wq_t[k][:, m * 128:(m + 1) * 128],
                                xqt[k][:], start=(k == 0), stop=(k == 7))
                        sc = scrq.tile([128, 512], F32R, tag="scr", name=f"scq{m}")
                        nc.vector.tensor_copy(sc[:], ps[:])
                        nc.sync.dma_start(out=QT[2 * m][0:64, :], in_=sc[0:64, :])
                        nc.sync.dma_start(
                            out=QT[2 * m + 1][0:64, :], in_=sc[64:128, :])

                # wk prefetch pool opens as soon as Q's pools free
                with tc.tile_pool(name="wkp", bufs=5) as wkp:
                    wk_t = [wkp.tile([128, D], F32R, tag="wk", name=f"wk{k}")
                            for k in range(5)]
                    for k in range(5):
                        nc.sync.dma_start(
                            out=wk_t[k][:], in_=wk[k * 128:(k + 1) * 128, :])

                    # ---- V (natural layout [s, d']) ----
                    with (
                        tc.tile_pool(name="wvp", bufs=8) as wvp,
                        tc.tile_pool(name="ppsV", bufs=4, space="PSUM") as ppsV,
                    ):
                        wv_t = [wvp.tile([128, D], F32R, tag="wv", name=f"wv{k}")
                                for k in range(8)]
                        for k in range(8):
                            nc.sync.dma_start(
                                out=wv_t[k][:], in_=wv[k * 128:(k + 1) * 128, :])
                        for m in range(8):
                            for n in range(2):
                                sl = slice(n * 512, (n + 1) * 512)
                                ps = ppsV.tile([128, 512], F32, tag="pps",
                                               name=f"psv{m}{n}")
                                for k in range(8):
                                    nc.tensor.matmul(
                                        ps[:], xt[k][:, m * 128:(m + 1) * 128],
                                        wv_t[k][:, sl], start=(k == 0), stop=(k == 7))
                                nc.vector.tensor_copy(V[m][:, sl], ps[:])

                    # ---- interleaved K projection + attention pipeline ----
                    with (
                        tc.tile_pool(name="wk7p", bufs=1) as wk7p,
                        tc.tile_pool(name="scrk", bufs=2) as scrk,
                        tc.tile_pool(name="pp", bufs=3) as ppool,
                        tc.tile_pool(name="ptp", bufs=6) as ptp,
                        tc.tile_pool(name="smp", bufs=2) as smp,
                        tc.tile_pool(name="ppsK", bufs=1, space="PSUM") as ppsK,
                        tc.tile_pool(name="psA", bufs=2, space="PSUM") as psA,
                        tc.tile_pool(name="psB", bufs=2, space="PSUM") as psB,
                        tc.tile_pool(name="psO", bufs=1, space="PSUM") as psO,
                    ):
                        wk_late = [wk7p.tile([128, D], F32R, name=f"wk{k}l")
                                   for k in (5, 6, 7)]
                        for i, k in enumerate((5, 6, 7)):
                            nc.sync.dma_start(
                                out=wk_late[i][:], in_=wk[k * 128:(k + 1) * 128, :])
                        wk_all = wk_t + wk_late
                        prev = None
                        for m in range(8):
                            if prev is not None:
                                _pair_BO(nc, tc, prev, KT, QT, V, OT, ptp, psB, psO)
                            for n in range(2):
                                sl = slice(n * 512, (n + 1) * 512)
                                ps = ppsK.tile([128, 512], F32, tag="ppsK",
                                               name=f"psk{m}{n}")
                                for k in range(8):
                                    nc.tensor.matmul(
                                        ps[:], wk_all[k][:, m * 128:(m + 1) * 128],
                                        xt[k][:, sl], start=(k == 0), stop=(k == 7))
                                sc = scrk.tile([128, 512], F32R, tag="scr",
                                               name=f"sck{m}{n}")
                                nc.vector.tensor_copy(sc[:], ps[:])
                                nc.sync.dma_start(
                                    out=KT[2 * m][0:64, sl], in_=sc[0:64, :])
                                nc.sync.dma_start(
                                    out=KT[2 * m + 1][0:64, sl], in_=sc[64:128, :])
                            _head_A(nc, tc, 2 * m, KT, QT, ppool, psA, smp, attw)
                            _head_A(nc, tc, 2 * m + 1, KT, QT, ppool, psA, smp, attw)
                            prev = m
                        _pair_BO(nc, tc, prev, KT, QT, V, OT, ptp, psB, psO)

            # ---- dense output projection (bf16) ----
            with (
                tc.tile_pool(name="wdp", bufs=8) as wdp,
                tc.tile_pool(name="dop", bufs=3) as dop,
                tc.tile_pool(name="psD", bufs=4, space="PSUM") as psD,
            ):
                wd_t = [wdp.tile([128, D], BF16, tag="wd", name=f"wd{k}")
                        for k in range(8)]
                for k in range(8):
                    nc.sync.dma_start(out=wd_t[k][:], in_=wd[k * 128:(k + 1) * 128, :])
                for m in range(4):
                    for n in range(2):
                        sl = slice(n * 512, (n + 1) * 512)
                        ps = psD.tile([128, 512], F32, tag="psD", name=f"psd{m}{n}")
                        for k in range(8):
                            nc.tensor.matmul(
                                ps[:],
                                OT[k][:, m * 128:(m + 1) * 128],
                                wd_t[k][:, sl],
                                start=(k == 0), stop=(k == 7))
                        ot_ = dop.tile([128, 512], F32, tag="do", name=f"do{m}{n}")
                        nc.vector.tensor_copy(ot_[:], ps[:])
                        nc.sync.dma_start(
                            out=atto[m * 128:(m + 1) * 128, sl], in_=ot_[:])

    nc.compile()
    _cache[("nc", nreps)] = nc
    return nc


def make_in_maps(x, attention_mask, Wq, Wk, Wv, Wb, mixing, Wd):
    x = np.asarray(x, dtype=np.float32)
    mixflat = np.asarray(mixing, dtype=np.float32).reshape(-1) / np.float32(np.sqrt(HD))
    wq_eff = np.ascontiguousarray(np.asarray(Wq, np.float32) * mixflat[None, :])
    wk_ = np.ascontiguousarray(np.asarray(Wk, np.float32))
    wv_ = np.ascontiguousarray(np.asarray(Wv, np.float32))
    wb_ = np.ascontiguousarray(np.asarray(Wb, np.float32))
    wd_ = np.ascontiguousarray(np.asarray(Wd, np.float32).astype(ml_dtypes.bfloat16))
    am = np.asarray(attention_mask, np.float32)
    in_maps = []
    for c in range(8):
        b, qh = c // 2, c % 2
        xTb = np.ascontiguousarray(x[b].T)
        in_maps.append({
            "xT": xTb,
            "xq": np.ascontiguousarray(xTb[:, qh * SQ:(qh + 1) * SQ]),
            "wq": wq_eff, "wk": wk_, "wv": wv_, "wb": wb_, "wd": wd_,
            "mask": np.ascontiguousarray(am[b, 0, 0, :].reshape(1, S)),
        })
    return in_maps


def kernel(x, attention_mask, Wq, Wk, Wv, Wb, mixing, Wd, bd, _trace=False):
    global LAST_RESULTS
    nc = build()
    in_maps = make_in_maps(x, attention_mask, Wq, Wk, Wv, Wb, mixing, Wd)
    res = run_bass_kernel_spmd(nc, in_maps, list(range(8)), trace=_trace)
    LAST_RESULTS = res
    attn_output = np.empty((B, S, D), np.float32)
    attn_weights = np.empty((B, H, S, S), np.float32)
    for c in range(8):
        b, qh = c // 2, c % 2
        r = res.results[c]
        attn_output[b, qh * SQ:(qh + 1) * SQ, :] = r["atto"]
        attn_weights[b, :, qh * SQ:(qh + 1) * SQ, :] = r["attw"]
    attn_output += np.asarray(bd, np.float32)[None, None, :]
    return attn_output, attn_weights
